# revision 61
# baseline (speedup 1.0000x reference)
"""Causal self-attention (B=4, T=2048, C=1024, H=16) on 8 TRN2 NeuronCores.

Sharding: core = (batch, head-group) on a 4x2 grid. Each core computes the
attention output of 8 heads for one batch element plus its partial out-proj
(y^T = w_out_slice^T @ out_heads^T); the two head-groups of a batch are summed
on the host (the "out_proj all-reduce"), where the final bias is also added.

On-chip dataflow is fully transposed so no transposes are ever needed:
  qk^T  = w_qkv_slice^T @ x^T          (C on partitions)
  v     = x @ w_v_slice                (T on partitions, natural)
  S^T   = k_h @ q_h^T                  (k-positions on partitions)
  P^T   = exp(S^T)                     (no max-subtraction: scores ~ N(0,1))
  outT  = [v|1]^T @ P^T                (ones column accumulates sum-of-exp)
  y^T   = w_out_slice^T @ (outT/sumexp)

v2 structure (vs the 414us baseline):
  - causal mask applied on the PE: a rank-structured accumulate matmul
    (TRI^T @ (-100*I)) adds -100 to the upper triangle of each diagonal
    score block, so exp() underflows to zero -- no DVE mask multiply.
  - diagonal blocks trim scores/exp/PV to the causally-valid query columns.
  - the S->exp->PV chain is software-pipelined: PV(i) is emitted two
    iterations behind S(i) so the PE never waits on the ACT engine.
  - projection matmuls of the NEXT head-pair (and out-proj chunks during the
    last pair) are interleaved into the attention loop as PE filler.
  - normalization: sum-of-exp rides the PV matmul (ones column); 1/sumexp is
    broadcast for BOTH heads with one ONES2 matmul; results are written into
    OT directly by the DVE (odd heads land on partitions 64-127 by shifting
    the PV stationary window so no partition-moving DMA is needed).
  - bulk input loads ride the scalar+vector DMA queues with 2-3KB lines;
    the sync queue carries only the small normalization DMAs + output stores.
  - y^T is stored as bf16 (host accumulates the pair of partials in f32).
"""

import sys
import types

if "/opt/trn_rl_repo" not in sys.path:
    sys.path.insert(0, "/opt/trn_rl_repo")

import numpy as np


def _install_ntff_hook_shim():
    """antenv.axon_hooks is missing in this image; provide it so that
    run_bass_kernel_spmd(trace=True) can capture NTFF profiles."""
    if "antenv.axon_hooks" in sys.modules:
        return
    try:
        from trn_agent_boot.trn_boot import _ntff_profile_via_ctypes

        hook = _ntff_profile_via_ctypes("/opt/axon/libaxon_pjrt.so")
    except Exception:
        hook = None
    m = types.ModuleType("antenv.axon_hooks")
    m.get_axon_ntff_profile_hook = lambda: hook
    sys.modules["antenv.axon_hooks"] = m


_install_ntff_hook_shim()

import concourse.bass as bass  # noqa: E402
from concourse import bacc  # noqa: E402
import concourse.mybir as mybir  # noqa: E402
import concourse.tile as tile  # noqa: E402
from concourse.bass_utils import run_bass_kernel_spmd  # noqa: E402

BF16 = mybir.dt.bfloat16
F32 = mybir.dt.float32
NPBF16 = mybir.dt.np(BF16)
EXP = mybir.ActivationFunctionType.Exp

B, T, C = 4, 2048, 1024
H, DH = 16, 64
HC = 8           # heads per core
CK = C // 128    # 8 contraction chunks over C
TB = T // 128    # 16 key blocks / T row blocks
QC = T // 512    # 4 query chunks
SCALE = 1.0 / np.sqrt(DH)
NEG = -100.0     # causal mask additive constant (exp underflows to 0)

TRACE = False          # set True (e.g. from test.py) to capture an NTFF profile
LAST_RESULT = None     # BassKernelResults of the last run (exec_time_ns etc.)

_CACHE = None


def _build(use_vbias=True):
    nc = bacc.Bacc("TRN2", target_bir_lowering=False, debug=False, num_devices=8)

    xT = nc.dram_tensor("xT", [C, T], BF16, kind="ExternalInput")
    wqkv = nc.dram_tensor("wqkv", [C, 3 * 512], BF16, kind="ExternalInput")
    bqk = nc.dram_tensor("bqk", [128, CK], F32, kind="ExternalInput")
    bv2 = nc.dram_tensor("bv2", [128, 4], F32, kind="ExternalInput")
    wout = nc.dram_tensor("wout", [512, C], BF16, kind="ExternalInput")
    tri = nc.dram_tensor("tri", [128, 128], BF16, kind="ExternalInput")
    negid = nc.dram_tensor("negid", [128, 128], BF16, kind="ExternalInput")
    ones2 = nc.dram_tensor("ones2", [128, 128], BF16, kind="ExternalInput")
    yT = nc.dram_tensor("yT", [C, T], BF16, kind="ExternalOutput")

    # v block row layout, per head-pair (193 cols): [v_even 64][ones][ones]
    # [pad 63][v_odd 64].  The even head's PV stationary window starts at the
    # pair base (v at out partitions 0-63, sum-of-exp at partition 64); the
    # odd head's window starts at its ones column (sum-of-exp at partition 0,
    # v at partitions 64-127) -- both legal DVE partition bases, so norms
    # write OT in place with no partition-moving DMA.  Pad columns hold 1.0
    # (harmless: they only feed unread output partitions).
    VROW = 4 * 193 + 129  # odd-head copy AP needs stride-193 slack past 772

    with tile.TileContext(nc) as tc:
        with (
            tc.tile_pool(name="persist", bufs=1) as pp,
            tc.tile_pool(name="sc", bufs=3, space="PSUM") as scp,
            tc.tile_pool(name="oa", bufs=2, space="PSUM") as oap,
            tc.tile_pool(name="pt", bufs=8) as ptp,
            tc.tile_pool(name="nrm", bufs=9) as nrm,
            tc.tile_pool(name="ocp", bufs=6) as ocpp,
            tc.tile_pool(name="yst", bufs=6) as yst,
        ):
            # q^T and k^T packed per pair: head h in partitions (h%2)*64..;
            # score matmuls contract over just that head's 64 partitions.
            QT = [pp.tile([128, T], BF16, tag=f"qt{p}", name=f"qt{p}")
                  for p in range(4)]
            KPP = [pp.tile([128, T], BF16, tag=f"kpp{p}", name=f"kpp{p}")
                   for p in range(4)]
            OT = [pp.tile([128, T], BF16, tag=f"ot{p}", name=f"ot{p}")
                  for p in range(4)]
            VA = pp.tile([128, TB, VROW], BF16, tag="va")
            WOUT = pp.tile([128, 4, C], BF16, tag="wout")
            BQK = pp.tile([128, CK], F32, tag="bqk")
            BV2 = pp.tile([128, 4], F32, tag="bv2")
            XT = pp.tile([128, CK, T], BF16, tag="xt")
            WQ = pp.tile([128, CK, 1536], BF16, tag="wq")
            TRI = pp.tile([128, 128], BF16, tag="tri")
            NEGID = pp.tile([128, 128], BF16, tag="negid")
            ONES2 = pp.tile([128, 128], BF16, tag="ones2")
            RRP = [pp.tile([128, 512], BF16, tag=f"rrp{i}", name=f"rrp{i}")
                   for i in range(3)]

            # ---- input DMA: bulk on sync+scalar queues, ordered by first
            # use so the v projection never waits.  All bulk lands well
            # before the norm smalls join the sync queue.
            for kc in range(CK):
                nc.sync.dma_start(
                    XT[:, kc, 0:1024], xT[kc * 128:(kc + 1) * 128, 0:1024]
                )
            for n in (2, 3):
                for kc in range(CK):
                    nc.sync.dma_start(
                        XT[:, kc, n * 512:(n + 1) * 512],
                        xT[kc * 128:(kc + 1) * 128, n * 512:(n + 1) * 512],
                    )
            # scalar queue: v weights (needed immediately), then q|k weights.
            for kc in range(CK):
                nc.scalar.dma_start(
                    WQ[:, kc, 1024:1536], wqkv[kc * 128:(kc + 1) * 128, 1024:1536]
                )
            for kc in range(CK):
                nc.scalar.dma_start(
                    WQ[:, kc, 0:1024], wqkv[kc * 128:(kc + 1) * 128, 0:1024]
                )
            # gpsimd queue: small constants now; the cold bulk (x n=3, wout)
            # issues after the memsets so it stays off the startup HBM window.
            nc.gpsimd.dma_start(TRI[:], tri[:])
            nc.gpsimd.dma_start(NEGID[:], negid[:])
            nc.gpsimd.dma_start(ONES2[:], ones2[:])
            nc.gpsimd.dma_start(BQK[:], bqk[:])
            nc.gpsimd.dma_start(BV2[:], bv2[:])

            # ones columns of VA; v copies below overwrite the v columns
            nc.gpsimd.memset(VA[:], 1.0)
            # rrp rows >=2 are multiplied by zero stationary columns, but must
            # not be Inf/NaN garbage -- zero the ring once.
            for i in range(3):
                nc.vector.memset(RRP[i][:], 0.0)

            for kc in range(4):
                nc.gpsimd.dma_start(WOUT[:, kc, :], wout[kc * 128:(kc + 1) * 128, :])

            # ---------------- emission helpers ----------------
            def emit_v(psl, t):
                for kc in range(CK):
                    nc.tensor.matmul(
                        psl,
                        XT[:, kc, t * 128:(t + 1) * 128],
                        WQ[:, kc, 1024:1536],
                        start=(kc == 0),
                        stop=(kc == CK - 1),
                    )
                src = psl.rearrange("p (g c) -> p g c", c=128)
                dste = VA[:, t, 0:772].rearrange("p (g c) -> p g c", c=193)
                dsto = VA[:, t, 129:901].rearrange("p (g c) -> p g c", c=193)
                nc.vector.tensor_copy(dste[:, :, 0:64], src[:, :, 0:64])
                nc.vector.tensor_copy(dsto[:, :, 0:64], src[:, :, 64:128])

            def emit_qk_half(psl, pair, qk, n, half):
                """One 4-kc half of a qk projection matmul group."""
                m = pair + 4 * qk
                for kc in range(4 * half, 4 * half + 4):
                    nc.tensor.matmul(
                        psl,
                        WQ[:, kc, m * 128:(m + 1) * 128],
                        XT[:, kc, n * 512:(n + 1) * 512],
                        start=(kc == 0),
                        stop=(kc == CK - 1),
                    )
                if half == 1:
                    ns = slice(n * 512, (n + 1) * 512)
                    dst = QT[pair] if qk == 0 else KPP[pair]
                    # ACT-engine identity+bias: same act table as exp (no
                    # reload), keeps the PSUM->SBUF move off the DVE queue
                    nc.scalar.add(dst[:, ns], psl, BQK[:, m:m + 1])

            # filler machinery: closures of ~1.7us of independent PE work,
            # one PSUM-tile allocation each (keeps scp pool rotation slack)
            fillers = []
            FILLER_RESERVE = [0]

            def add_qkproj_fillers(pair):
                jobs = [(qk, n) for qk in range(2) for n in range(4)]
                for g0 in range(0, 8, 2):
                    # one psum tile per 2 jobs, emitted as 4 quarter-fillers
                    state = {}

                    def mk(g0=g0, state=state):
                        def fill0():
                            state["t"] = scp.tile([128, 1024], F32, tag="sc",
                                                  name="qkps")
                            qk, n = jobs[g0]
                            emit_qk_half(state["t"][:, 0:512], pair, qk, n, 0)

                        def fill1():
                            qk, n = jobs[g0]
                            emit_qk_half(state["t"][:, 0:512], pair, qk, n, 1)

                        def fill2():
                            qk, n = jobs[g0 + 1]
                            emit_qk_half(state["t"][:, 512:1024], pair, qk, n, 0)

                        def fill3():
                            qk, n = jobs[g0 + 1]
                            emit_qk_half(state["t"][:, 512:1024], pair, qk, n, 1)

                        return [fill0, fill1, fill2, fill3]

                    fillers.extend(mk())

            def add_vproj_fillers():
                for t2 in range(8, TB, 2):
                    state = {}

                    def mk(t2=t2, state=state):
                        def fill0():
                            state["t"] = scp.tile([128, 1024], F32, tag="sc",
                                                  name="vps")
                            emit_v(state["t"][:, 0:512], t2)

                        def fill1():
                            emit_v(state["t"][:, 512:1024], t2 + 1)

                        return [fill0, fill1]

                    fillers.extend(mk())

            def emit_ymm(psl, mo, n):
                for kc in range(4):
                    nc.tensor.matmul(
                        psl,
                        WOUT[:, kc, mo * 128:(mo + 1) * 128],
                        OT[kc][:, n * 512:(n + 1) * 512],
                        start=(kc == 0),
                        stop=(kc == 3),
                    )

            def add_yjob_fillers(n):
                for mo2 in range(0, 8, 2):
                    state = {}

                    def mk(mo2=mo2, n=n, state=state):
                        def fill0():
                            state["t"] = scp.tile([128, 1024], F32, tag="sc",
                                                  name="yps")
                            emit_ymm(state["t"][:, 0:512], mo2, n)

                        def fill1():
                            emit_ymm(state["t"][:, 512:1024], mo2 + 1, n)
                            ys = yst.tile([128, 1024], BF16, tag="ys",
                                          name="ys")
                            nc.vector.tensor_copy(ys[:], state["t"][:])
                            ys3 = ys[:].rearrange("p (g c) -> p g c", g=2)
                            dst = yT[mo2 * 128:(mo2 + 2) * 128,
                                     n * 512:(n + 1) * 512]
                            dst3 = dst.rearrange("(g p) c -> p g c", g=2)
                            nc.sync.dma_start(dst3, ys3)

                        return [fill0, fill1]

                    fillers.extend(mk())

            def pop_filler():
                if len(fillers) > FILLER_RESERVE[0]:
                    fillers.pop(0)()

            # ---- normalization ----
            pending_norms = []
            _rrp_ctr = [0]

            def norm_part1(pair, j, oaccs):
                """Right after the last PV of (pair, j): extract sum-of-exp for
                both heads, compute 1/se spread over 64 partitions, and land
                the two bf16 reciprocal rows in an RRP ring tile."""
                rrp = RRP[_rrp_ctr[0] % 3]
                _rrp_ctr[0] += 1
                ocps = {}
                # reciprocal chain first: it gates the deferred broadcast
                # matmul, so it must not queue behind the ocp copies.
                for s, h in enumerate((2 * pair, 2 * pair + 1)):
                    oacc = oaccs[h]
                    serow = 64 * (1 - h % 2)     # ones row: 64 (even)/0 (odd)
                    rc = nrm.tile([128, 512], F32, tag="rc", name="rc")
                    nc.vector.tensor_copy(rc[serow:serow + 1, :],
                                          oacc[serow:serow + 1, :])
                    rs = nrm.tile([64, 8], F32, tag="rs", name="rs")
                    # scalar-engine DMA ring: empty after the prologue, so
                    # these latency-critical hops never queue behind stores
                    nc.scalar.dma_start(out=rs[:], in_=rc[serow:serow + 1, :])
                    rsb = nrm.tile([64, 8], BF16, tag="rsb", name="rsb")
                    with nc.allow_low_precision("1/sumexp feeds a bf16 matmul"):
                        nc.vector.reciprocal(rsb[:], rs[:])
                    nc.scalar.dma_start(out=rrp[s:s + 1, :], in_=rsb[:])
                for s, h in enumerate((2 * pair, 2 * pair + 1)):
                    po = (h % 2) * 64            # out rows live at po..po+64
                    ocp = ocpp.tile([128, 512], F32, tag="ocp", name="ocp")
                    nc.vector.tensor_copy(ocp[po:po + 64, :],
                                          oaccs[h][po:po + 64, :])
                    ocps[h] = ocp
                return ocps, rrp

            def norm_part2(pair, j, ocps, rrp):
                """Deferred: broadcast 1/se for both heads with one matmul
                (rows 0-63 <- rrp row 0, rows 64-127 <- rrp row 1), scale,
                add bias, write straight into OT at the right partitions."""
                bct = scp.tile([128, 1024], F32, tag="sc", name="bc")
                bc = bct[:, 0:512]
                nc.tensor.matmul(bc, ONES2[:], rrp[:], start=True, stop=True)
                jc = slice(j * 512, (j + 1) * 512)
                for h in (2 * pair, 2 * pair + 1):
                    po = (h % 2) * 64
                    sl = slice(po, po + 64)
                    nc.vector.tensor_mul(OT[pair][sl, jc], ocps[h][sl, :],
                                         bc[sl, :])
                    if use_vbias:
                        nc.vector.tensor_scalar_add(OT[pair][sl, jc],
                                                    OT[pair][sl, jc],
                                                    BV2[sl, pair:pair + 1])

            def flush_norms():
                while pending_norms:
                    pending_norms.pop(0)()

            # ---- attention j-loop as a schedulable unit; the S->PV chain is
            # software-pipelined lag-3 WITHIN a loop and the first 3 S-groups
            # of the next loop overlap the current loop's last 3 PVs, so the
            # pipeline never drains at (pair, j) boundaries. ----
            class Attn:
                def __init__(self, pair, j):
                    self.pair = pair
                    self.j = j
                    self.heads = (2 * pair, 2 * pair + 1)
                    self.nb = 4 * (j + 1)
                    self.oaccs = None
                    self.pttile = [None] * self.nb

                def emit_s(self, i):
                    pair, j = self.pair, self.j
                    d = i - 4 * j
                    sc = scp.tile([128, 1024], F32, tag="sc", name="sc")
                    lo = max(d, 0) * 128
                    for s, h in enumerate(self.heads):
                        po = (h % 2) * 64
                        nc.tensor.matmul(
                            sc[:, s * 512 + lo:(s + 1) * 512],
                            KPP[pair][po:po + 64, i * 128:(i + 1) * 128],
                            QT[pair][po:po + 64, j * 512 + lo:(j + 1) * 512],
                            start=True,
                            stop=(d < 0),
                        )
                    if d >= 0:
                        # causal mask on the PE: add -100 to the strict upper
                        # triangle of the diagonal 128x128 sub-block
                        for s in range(2):
                            nc.tensor.matmul(
                                sc[:, s * 512 + d * 128:s * 512 + (d + 1) * 128],
                                TRI[:],
                                NEGID[:],
                                start=False,
                                stop=True,
                            )
                    pt = ptp.tile([128, 1024], BF16, tag="pt")
                    self.pttile[i] = pt
                    if lo == 0:
                        nc.scalar.activation(pt[:], sc[:], EXP)
                    else:
                        sc3 = sc[:].rearrange("p (s w) -> p s w", s=2)
                        pt3 = pt[:].rearrange("p (s w) -> p s w", s=2)
                        nc.scalar.activation(pt3[:, :, lo:512], sc3[:, :, lo:512],
                                             EXP)

                def emit_pv(self, i):
                    if self.oaccs is None:
                        self.oaccs = {
                            h: oap.tile([128, 512], F32, tag="oacc",
                                        name=f"oacc{h}")
                            for h in self.heads
                        }
                    d = i - 4 * self.j
                    lo = max(d, 0) * 128
                    pt = self.pttile[i]
                    for s, h in enumerate(self.heads):
                        vb = (h // 2) * 193 + (h % 2) * 65
                        nc.tensor.matmul(
                            self.oaccs[h][:, lo:512],
                            VA[:, i, vb:vb + 128],
                            pt[:, s * 512 + lo:(s + 1) * 512],
                            start=(i == 0),
                            stop=(i == self.nb - 1),
                            skip_group_check=True,
                        )
                    self.pttile[i] = None

            def run_attention(cur, nxt, start_i, after_flush=None):
                nb = cur.nb
                flush_i = 3 if nb == 4 else 6
                for i in range(start_i, nb):
                    cur.emit_s(i)
                    if i == flush_i:
                        flush_norms()
                        if after_flush is not None:
                            after_flush()
                    elif i % 2 == 1 or len(fillers) - FILLER_RESERVE[0] > 8:
                        pop_filler()
                    if i >= 3:
                        cur.emit_pv(i - 3)
                for t, pvi in enumerate((nb - 3, nb - 2, nb - 1)):
                    if nxt is not None:
                        nxt.emit_s(t)
                    else:
                        pop_filler()
                    cur.emit_pv(pvi)
                return norm_part1(cur.pair, cur.j, cur.oaccs)

            # ---------------- main schedule ----------------
            # prologue: v projection t0-7 + pair-0 qk projection; v t8-15
            # rides pair 0's attention as filler (first needed at j=2, i=8)
            for t2 in range(0, 8, 2):
                t3 = scp.tile([128, 1024], F32, tag="sc", name="vps")
                emit_v(t3[:, 0:512], t2)
                emit_v(t3[:, 512:1024], t2 + 1)

            add_qkproj_fillers(0)
            while fillers:
                pop_filler()

            loops = [Attn(pair, j) for pair in range(4) for j in range(QC)]
            for idx, cur in enumerate(loops):
                pair, j = cur.pair, cur.j
                if pair == 0 and j == 0:
                    add_vproj_fillers()
                if pair < 3 and j == 0:
                    add_qkproj_fillers(pair + 1)
                # out-proj chunk n becomes legal once pair3's j=n norm is
                # flushed; that happens at the flush point of (pair3, j+1)
                hook = (
                    (lambda j=j: add_yjob_fillers(j - 1))
                    if (pair == 3 and j >= 1) else None
                )
                if pair == 3 and j == 3:
                    # hold back two out-proj fillers: they give the PE
                    # work while the final norm's reciprocal chain runs
                    FILLER_RESERVE[0] = 2
                nxt = loops[idx + 1] if idx + 1 < len(loops) else None
                ocps, rrp = run_attention(cur, nxt, 0 if idx == 0 else 3,
                                          after_flush=hook)
                pending_norms.append(
                    lambda pair=pair, j=j, ocps=ocps, rrp=rrp:
                    norm_part2(pair, j, ocps, rrp)
                )

            FILLER_RESERVE[0] = 0
            while fillers:
                pop_filler()
            flush_norms()
            add_yjob_fillers(3)
            while fillers:
                pop_filler()

    nc.compile()
    return nc


def kernel(x, w_qkv, b_qkv, w_out, b_out):
    global _CACHE, LAST_RESULT
    x = np.asarray(x, np.float32)
    w_qkv = np.asarray(w_qkv, np.float32)
    b_qkv = np.asarray(b_qkv, np.float32)
    w_out = np.asarray(w_out, np.float32)
    b_out = np.asarray(b_out, np.float32)

    # the v-bias add costs 32 DVE ops per core; build without it when the
    # bias is identically zero (rebuilds if ever called with a real bias)
    use_vbias = bool(np.any(b_qkv[2048:3072]))
    if _CACHE is None or _CACHE[0] != use_vbias:
        _CACHE = (use_vbias, _build(use_vbias))
    nc = _CACHE[1]

    tri_c = np.triu(np.ones((128, 128), np.float32), 1).astype(NPBF16)
    negid_c = (NEG * np.eye(128, dtype=np.float32)).astype(NPBF16)
    ones2_c = np.zeros((128, 128), np.float32)
    ones2_c[0, 0:64] = 1.0
    ones2_c[1, 64:128] = 1.0
    ones2_c = ones2_c.astype(NPBF16)

    in_maps = []
    for core in range(8):
        b = core // 2
        g = core % 2
        sl = slice(g * 512, (g + 1) * 512)
        wq = w_qkv[:, 0:1024][:, sl] * SCALE
        wk = w_qkv[:, 1024:2048][:, sl]
        wv = w_qkv[:, 2048:3072][:, sl]
        wqkv_c = np.ascontiguousarray(
            np.concatenate([wq, wk, wv], axis=1).astype(NPBF16)
        )
        bq = b_qkv[0:1024][sl] * SCALE
        bk = b_qkv[1024:2048][sl]
        bqk_c = np.ascontiguousarray(
            np.concatenate([bq, bk]).reshape(CK, 128).T.astype(np.float32)
        )
        # bv2[p, pair] = v-bias of head (2*pair + p//64), dim p%64
        bv_core = b_qkv[2048:3072][sl].reshape(HC, 64)
        bv2_c = np.ascontiguousarray(
            bv_core.reshape(4, 2, 64).transpose(0, 1, 2).reshape(4, 128).T
            .astype(np.float32)
        )
        in_maps.append(
            {
                "xT": np.ascontiguousarray(x[b].T.astype(NPBF16)),
                "wqkv": wqkv_c,
                "bqk": bqk_c,
                "bv2": bv2_c,
                "wout": np.ascontiguousarray(w_out[sl, :].astype(NPBF16)),
                "tri": tri_c,
                "negid": negid_c,
                "ones2": ones2_c,
            }
        )

    res = run_bass_kernel_spmd(nc, in_maps, core_ids=list(range(8)), trace=TRACE)
    LAST_RESULT = res

    out = np.empty((B, T, C), np.float32)
    for b in range(B):
        acc = res.results[2 * b]["yT"].astype(np.float32) + res.results[
            2 * b + 1
        ]["yT"].astype(np.float32)
        out[b] = acc.T + b_out[None, :]
    return out


# revision 62
# speedup vs baseline: 1.0197x; 1.0197x over previous
"""Causal self-attention (B=4, T=2048, C=1024, H=16) on 8 TRN2 NeuronCores.

Sharding: core = (batch, head-group) on a 4x2 grid. Each core computes the
attention output of 8 heads for one batch element plus its partial out-proj
(y^T = w_out_slice^T @ out_heads^T); the two head-groups of a batch are summed
on the host (the "out_proj all-reduce"), where the final bias is also added.

On-chip dataflow is fully transposed so no transposes are ever needed:
  qk^T  = w_qkv_slice^T @ x^T          (C on partitions)
  v     = x @ w_v_slice                (T on partitions, natural)
  S^T   = k_h @ q_h^T                  (k-positions on partitions)
  P^T   = exp(S^T)                     (no max-subtraction: scores ~ N(0,1))
  outT  = [v|1]^T @ P^T                (ones column accumulates sum-of-exp)
  y^T   = w_out_slice^T @ (outT/sumexp)

v2 structure (vs the 414us baseline):
  - causal mask applied on the PE: a rank-structured accumulate matmul
    (TRI^T @ (-100*I)) adds -100 to the upper triangle of each diagonal
    score block, so exp() underflows to zero -- no DVE mask multiply.
  - diagonal blocks trim scores/exp/PV to the causally-valid query columns.
  - the S->exp->PV chain is software-pipelined: PV(i) is emitted two
    iterations behind S(i) so the PE never waits on the ACT engine.
  - projection matmuls of the NEXT head-pair (and out-proj chunks during the
    last pair) are interleaved into the attention loop as PE filler.
  - normalization: sum-of-exp rides the PV matmul (ones column); 1/sumexp is
    broadcast for BOTH heads with one ONES2 matmul; results are written into
    OT directly by the DVE (odd heads land on partitions 64-127 by shifting
    the PV stationary window so no partition-moving DMA is needed).
  - bulk input loads ride the scalar+vector DMA queues with 2-3KB lines;
    the sync queue carries only the small normalization DMAs + output stores.
  - y^T is stored as bf16 (host accumulates the pair of partials in f32).
"""

import sys
import types

if "/opt/trn_rl_repo" not in sys.path:
    sys.path.insert(0, "/opt/trn_rl_repo")

import numpy as np


def _install_ntff_hook_shim():
    """antenv.axon_hooks is missing in this image; provide it so that
    run_bass_kernel_spmd(trace=True) can capture NTFF profiles."""
    if "antenv.axon_hooks" in sys.modules:
        return
    try:
        from trn_agent_boot.trn_boot import _ntff_profile_via_ctypes

        hook = _ntff_profile_via_ctypes("/opt/axon/libaxon_pjrt.so")
    except Exception:
        hook = None
    m = types.ModuleType("antenv.axon_hooks")
    m.get_axon_ntff_profile_hook = lambda: hook
    sys.modules["antenv.axon_hooks"] = m


_install_ntff_hook_shim()

import concourse.bass as bass  # noqa: E402
from concourse import bacc  # noqa: E402
import concourse.mybir as mybir  # noqa: E402
import concourse.tile as tile  # noqa: E402
from concourse.bass_utils import run_bass_kernel_spmd  # noqa: E402

BF16 = mybir.dt.bfloat16
F32 = mybir.dt.float32
NPBF16 = mybir.dt.np(BF16)
EXP = mybir.ActivationFunctionType.Exp

B, T, C = 4, 2048, 1024
H, DH = 16, 64
HC = 8           # heads per core
CK = C // 128    # 8 contraction chunks over C
TB = T // 128    # 16 key blocks / T row blocks
QC = T // 512    # 4 query chunks
SCALE = 1.0 / np.sqrt(DH)
NEG = -100.0     # causal mask additive constant (exp underflows to 0)

TRACE = False          # set True (e.g. from test.py) to capture an NTFF profile
LAST_RESULT = None     # BassKernelResults of the last run (exec_time_ns etc.)

_CACHE = None


def _build(use_vbias=True):
    nc = bacc.Bacc("TRN2", target_bir_lowering=False, debug=False, num_devices=8)

    xT = nc.dram_tensor("xT", [C, T], BF16, kind="ExternalInput")
    wqkv = nc.dram_tensor("wqkv", [C, 3 * 512], BF16, kind="ExternalInput")
    bqk = nc.dram_tensor("bqk", [128, CK], F32, kind="ExternalInput")
    bv2 = nc.dram_tensor("bv2", [128, 4], F32, kind="ExternalInput")
    wout = nc.dram_tensor("wout", [512, C], BF16, kind="ExternalInput")
    tri = nc.dram_tensor("tri", [128, 128], BF16, kind="ExternalInput")
    negid = nc.dram_tensor("negid", [128, 128], BF16, kind="ExternalInput")
    ones2 = nc.dram_tensor("ones2", [128, 128], BF16, kind="ExternalInput")
    yT = nc.dram_tensor("yT", [C, T], BF16, kind="ExternalOutput")

    # v block row layout, per head-pair (193 cols): [v_even 64][ones][ones]
    # [pad 63][v_odd 64].  The even head's PV stationary window starts at the
    # pair base (v at out partitions 0-63, sum-of-exp at partition 64); the
    # odd head's window starts at its ones column (sum-of-exp at partition 0,
    # v at partitions 64-127) -- both legal DVE partition bases, so norms
    # write OT in place with no partition-moving DMA.  Pad columns hold 1.0
    # (harmless: they only feed unread output partitions).
    VROW = 4 * 193 + 129  # odd-head copy AP needs stride-193 slack past 772

    with tile.TileContext(nc) as tc:
        with (
            tc.tile_pool(name="persist", bufs=1) as pp,
            tc.tile_pool(name="sc", bufs=3, space="PSUM") as scp,
            tc.tile_pool(name="oa", bufs=2, space="PSUM") as oap,
            tc.tile_pool(name="pt", bufs=8) as ptp,
            tc.tile_pool(name="nrm", bufs=9) as nrm,
            tc.tile_pool(name="ocp", bufs=6) as ocpp,
            tc.tile_pool(name="yst", bufs=6) as yst,
        ):
            # q^T and k^T packed per pair: head h in partitions (h%2)*64..;
            # score matmuls contract over just that head's 64 partitions.
            QT = [pp.tile([128, T], BF16, tag=f"qt{p}", name=f"qt{p}")
                  for p in range(4)]
            KPP = [pp.tile([128, T], BF16, tag=f"kpp{p}", name=f"kpp{p}")
                   for p in range(4)]
            OT = [pp.tile([128, T], BF16, tag=f"ot{p}", name=f"ot{p}")
                  for p in range(4)]
            VA = pp.tile([128, TB, VROW], BF16, tag="va")
            WOUT = pp.tile([128, 4, C], BF16, tag="wout")
            BQK = pp.tile([128, CK], F32, tag="bqk")
            BV2 = pp.tile([128, 4], F32, tag="bv2")
            XT = pp.tile([128, CK, T], BF16, tag="xt")
            WQ = pp.tile([128, CK, 1536], BF16, tag="wq")
            TRI = pp.tile([128, 128], BF16, tag="tri")
            NEGID = pp.tile([128, 128], BF16, tag="negid")
            ONES2 = pp.tile([128, 128], BF16, tag="ones2")
            RRP = [pp.tile([128, 512], BF16, tag=f"rrp{i}", name=f"rrp{i}")
                   for i in range(3)]

            # ---- input DMA: bulk on sync+scalar queues, ordered by first
            # use so the v projection never waits.  All bulk lands well
            # before the norm smalls join the sync queue.
            for kc in range(CK):
                nc.sync.dma_start(
                    XT[:, kc, 0:1024], xT[kc * 128:(kc + 1) * 128, 0:1024]
                )
            for n in (2, 3):
                for kc in range(CK):
                    nc.sync.dma_start(
                        XT[:, kc, n * 512:(n + 1) * 512],
                        xT[kc * 128:(kc + 1) * 128, n * 512:(n + 1) * 512],
                    )
            # scalar queue: v weights (needed immediately), then q|k weights.
            for kc in range(CK):
                nc.scalar.dma_start(
                    WQ[:, kc, 1024:1536], wqkv[kc * 128:(kc + 1) * 128, 1024:1536]
                )
            for kc in range(CK):
                nc.scalar.dma_start(
                    WQ[:, kc, 0:1024], wqkv[kc * 128:(kc + 1) * 128, 0:1024]
                )
            # gpsimd queue: small constants now; the cold bulk (x n=3, wout)
            # issues after the memsets so it stays off the startup HBM window.
            nc.gpsimd.dma_start(TRI[:], tri[:])
            nc.gpsimd.dma_start(NEGID[:], negid[:])
            nc.gpsimd.dma_start(ONES2[:], ones2[:])
            nc.gpsimd.dma_start(BQK[:], bqk[:])
            nc.gpsimd.dma_start(BV2[:], bv2[:])

            # ones columns of VA; v copies below overwrite the v columns
            nc.gpsimd.memset(VA[:], 1.0)
            # rrp rows >=2 are multiplied by zero stationary columns, but must
            # not be Inf/NaN garbage -- zero the ring once.
            for i in range(3):
                nc.vector.memset(RRP[i][:], 0.0)

            for kc in range(4):
                nc.gpsimd.dma_start(WOUT[:, kc, :], wout[kc * 128:(kc + 1) * 128, :])

            # ---------------- emission helpers ----------------
            def emit_v(psl, t):
                for kc in range(CK):
                    nc.tensor.matmul(
                        psl,
                        XT[:, kc, t * 128:(t + 1) * 128],
                        WQ[:, kc, 1024:1536],
                        start=(kc == 0),
                        stop=(kc == CK - 1),
                    )
                src = psl.rearrange("p (g c) -> p g c", c=128)
                dste = VA[:, t, 0:772].rearrange("p (g c) -> p g c", c=193)
                dsto = VA[:, t, 129:901].rearrange("p (g c) -> p g c", c=193)
                nc.vector.tensor_copy(dste[:, :, 0:64], src[:, :, 0:64])
                nc.vector.tensor_copy(dsto[:, :, 0:64], src[:, :, 64:128])

            def emit_qk_half(psl, pair, qk, n, half):
                """One 4-kc half of a qk projection matmul group."""
                m = pair + 4 * qk
                for kc in range(4 * half, 4 * half + 4):
                    nc.tensor.matmul(
                        psl,
                        WQ[:, kc, m * 128:(m + 1) * 128],
                        XT[:, kc, n * 512:(n + 1) * 512],
                        start=(kc == 0),
                        stop=(kc == CK - 1),
                    )
                if half == 1:
                    ns = slice(n * 512, (n + 1) * 512)
                    dst = QT[pair] if qk == 0 else KPP[pair]
                    # ACT-engine identity+bias: same act table as exp (no
                    # reload), keeps the PSUM->SBUF move off the DVE queue
                    nc.scalar.add(dst[:, ns], psl, BQK[:, m:m + 1])

            # filler machinery: closures of ~1.7us of independent PE work,
            # one PSUM-tile allocation each (keeps scp pool rotation slack)
            fillers = []
            FILLER_RESERVE = [0]

            def add_qkproj_fillers(pair):
                jobs = [(qk, n) for qk in range(2) for n in range(4)]
                for g0 in range(0, 8, 2):
                    # one psum tile per 2 jobs, emitted as 4 quarter-fillers
                    state = {}

                    def mk(g0=g0, state=state):
                        def fill0():
                            state["t"] = scp.tile([128, 1024], F32, tag="sc",
                                                  name="qkps")
                            qk, n = jobs[g0]
                            emit_qk_half(state["t"][:, 0:512], pair, qk, n, 0)

                        def fill1():
                            qk, n = jobs[g0]
                            emit_qk_half(state["t"][:, 0:512], pair, qk, n, 1)

                        def fill2():
                            qk, n = jobs[g0 + 1]
                            emit_qk_half(state["t"][:, 512:1024], pair, qk, n, 0)

                        def fill3():
                            qk, n = jobs[g0 + 1]
                            emit_qk_half(state["t"][:, 512:1024], pair, qk, n, 1)

                        return [fill0, fill1, fill2, fill3]

                    fillers.extend(mk())

            def add_vproj_fillers():
                for t2 in range(8, TB, 2):
                    state = {}

                    def mk(t2=t2, state=state):
                        def fill0():
                            state["t"] = scp.tile([128, 1024], F32, tag="sc",
                                                  name="vps")
                            emit_v(state["t"][:, 0:512], t2)

                        def fill1():
                            emit_v(state["t"][:, 512:1024], t2 + 1)

                        return [fill0, fill1]

                    fillers.extend(mk())

            def emit_ymm(psl, mo, n):
                for kc in range(4):
                    nc.tensor.matmul(
                        psl,
                        WOUT[:, kc, mo * 128:(mo + 1) * 128],
                        OT[kc][:, n * 512:(n + 1) * 512],
                        start=(kc == 0),
                        stop=(kc == 3),
                    )

            def add_yjob_fillers(n):
                for mo2 in range(0, 8, 2):
                    state = {}

                    def mk(mo2=mo2, n=n, state=state):
                        def fill0():
                            state["t"] = scp.tile([128, 1024], F32, tag="sc",
                                                  name="yps")
                            emit_ymm(state["t"][:, 0:512], mo2, n)

                        def fill1():
                            emit_ymm(state["t"][:, 512:1024], mo2 + 1, n)
                            ys = yst.tile([128, 1024], BF16, tag="ys",
                                          name="ys")
                            nc.vector.tensor_copy(ys[:], state["t"][:])
                            ys3 = ys[:].rearrange("p (g c) -> p g c", g=2)
                            dst = yT[mo2 * 128:(mo2 + 2) * 128,
                                     n * 512:(n + 1) * 512]
                            dst3 = dst.rearrange("(g p) c -> p g c", g=2)
                            nc.sync.dma_start(dst3, ys3)

                        return [fill0, fill1]

                    fillers.extend(mk())

            def pop_filler():
                if len(fillers) > FILLER_RESERVE[0]:
                    fillers.pop(0)()

            # ---- normalization ----
            pending_norms = []
            _rrp_ctr = [0]

            def norm_part1(pair, j, oaccs):
                """Right after the last PV of (pair, j): extract sum-of-exp for
                both heads, compute 1/se spread over 64 partitions, and land
                the two bf16 reciprocal rows in an RRP ring tile."""
                rrp = RRP[_rrp_ctr[0] % 3]
                _rrp_ctr[0] += 1
                ocps = {}
                # reciprocal chain first: it gates the deferred broadcast
                # matmul, so it must not queue behind the ocp copies.
                for s, h in enumerate((2 * pair, 2 * pair + 1)):
                    oacc = oaccs[h]
                    serow = 64 * (1 - h % 2)     # ones row: 64 (even)/0 (odd)
                    rc = nrm.tile([128, 512], F32, tag="rc", name="rc")
                    nc.vector.tensor_copy(rc[serow:serow + 1, :],
                                          oacc[serow:serow + 1, :])
                    rs = nrm.tile([64, 8], F32, tag="rs", name="rs")
                    nc.sync.dma_start(out=rs[:], in_=rc[serow:serow + 1, :])
                    rsb = nrm.tile([64, 8], BF16, tag="rsb", name="rsb")
                    with nc.allow_low_precision("1/sumexp feeds a bf16 matmul"):
                        nc.vector.reciprocal(rsb[:], rs[:])
                    nc.sync.dma_start(out=rrp[s:s + 1, :], in_=rsb[:])
                for s, h in enumerate((2 * pair, 2 * pair + 1)):
                    po = (h % 2) * 64            # out rows live at po..po+64
                    ocp = ocpp.tile([128, 512], F32, tag="ocp", name="ocp")
                    nc.vector.tensor_copy(ocp[po:po + 64, :],
                                          oaccs[h][po:po + 64, :])
                    ocps[h] = ocp
                return ocps, rrp

            def norm_part2(pair, j, ocps, rrp):
                """Deferred: broadcast 1/se for both heads with one matmul
                (rows 0-63 <- rrp row 0, rows 64-127 <- rrp row 1), scale,
                add bias, write straight into OT at the right partitions."""
                bct = scp.tile([128, 1024], F32, tag="sc", name="bc")
                bc = bct[:, 0:512]
                nc.tensor.matmul(bc, ONES2[:], rrp[:], start=True, stop=True)
                jc = slice(j * 512, (j + 1) * 512)
                for h in (2 * pair, 2 * pair + 1):
                    po = (h % 2) * 64
                    sl = slice(po, po + 64)
                    nc.vector.tensor_mul(OT[pair][sl, jc], ocps[h][sl, :],
                                         bc[sl, :])
                    if use_vbias:
                        nc.vector.tensor_scalar_add(OT[pair][sl, jc],
                                                    OT[pair][sl, jc],
                                                    BV2[sl, pair:pair + 1])

            def flush_norms():
                while pending_norms:
                    pending_norms.pop(0)()

            # ---- attention j-loop as a schedulable unit; the S->PV chain is
            # software-pipelined lag-3 WITHIN a loop and the first 3 S-groups
            # of the next loop overlap the current loop's last 3 PVs, so the
            # pipeline never drains at (pair, j) boundaries. ----
            class Attn:
                def __init__(self, pair, j):
                    self.pair = pair
                    self.j = j
                    self.heads = (2 * pair, 2 * pair + 1)
                    self.nb = 4 * (j + 1)
                    self.oaccs = None
                    self.pttile = [None] * self.nb

                def emit_s(self, i):
                    pair, j = self.pair, self.j
                    d = i - 4 * j
                    sc = scp.tile([128, 1024], F32, tag="sc", name="sc")
                    lo = max(d, 0) * 128
                    for s, h in enumerate(self.heads):
                        po = (h % 2) * 64
                        nc.tensor.matmul(
                            sc[:, s * 512 + lo:(s + 1) * 512],
                            KPP[pair][po:po + 64, i * 128:(i + 1) * 128],
                            QT[pair][po:po + 64, j * 512 + lo:(j + 1) * 512],
                            start=True,
                            stop=(d < 0),
                        )
                    if d >= 0:
                        # causal mask on the PE: add -100 to the strict upper
                        # triangle of the diagonal 128x128 sub-block
                        for s in range(2):
                            nc.tensor.matmul(
                                sc[:, s * 512 + d * 128:s * 512 + (d + 1) * 128],
                                TRI[:],
                                NEGID[:],
                                start=False,
                                stop=True,
                            )
                    pt = ptp.tile([128, 1024], BF16, tag="pt")
                    self.pttile[i] = pt
                    if lo == 0:
                        nc.scalar.activation(pt[:], sc[:], EXP)
                    else:
                        sc3 = sc[:].rearrange("p (s w) -> p s w", s=2)
                        pt3 = pt[:].rearrange("p (s w) -> p s w", s=2)
                        nc.scalar.activation(pt3[:, :, lo:512], sc3[:, :, lo:512],
                                             EXP)

                def emit_pv(self, i):
                    if self.oaccs is None:
                        self.oaccs = {
                            h: oap.tile([128, 512], F32, tag="oacc",
                                        name=f"oacc{h}")
                            for h in self.heads
                        }
                    d = i - 4 * self.j
                    lo = max(d, 0) * 128
                    pt = self.pttile[i]
                    for s, h in enumerate(self.heads):
                        vb = (h // 2) * 193 + (h % 2) * 65
                        nc.tensor.matmul(
                            self.oaccs[h][:, lo:512],
                            VA[:, i, vb:vb + 128],
                            pt[:, s * 512 + lo:(s + 1) * 512],
                            start=(i == 0),
                            stop=(i == self.nb - 1),
                            skip_group_check=True,
                        )
                    self.pttile[i] = None

            def run_attention(cur, nxt, start_i, after_flush=None):
                nb = cur.nb
                flush_i = 3 if nb == 4 else 6
                for i in range(start_i, nb):
                    cur.emit_s(i)
                    if i == flush_i:
                        flush_norms()
                        if after_flush is not None:
                            after_flush()
                    elif i % 2 == 1 or len(fillers) - FILLER_RESERVE[0] > 8:
                        pop_filler()
                    if i >= 3:
                        cur.emit_pv(i - 3)
                for t, pvi in enumerate((nb - 3, nb - 2, nb - 1)):
                    if nxt is not None:
                        nxt.emit_s(t)
                    else:
                        pop_filler()
                    cur.emit_pv(pvi)
                return norm_part1(cur.pair, cur.j, cur.oaccs)

            # ---------------- main schedule ----------------
            # prologue: v projection t0-7 + pair-0 qk projection; v t8-15
            # rides pair 0's attention as filler (first needed at j=2, i=8)
            for t2 in range(0, 8, 2):
                t3 = scp.tile([128, 1024], F32, tag="sc", name="vps")
                emit_v(t3[:, 0:512], t2)
                emit_v(t3[:, 512:1024], t2 + 1)

            add_qkproj_fillers(0)
            while fillers:
                pop_filler()

            loops = [Attn(pair, j) for pair in range(4) for j in range(QC)]
            for idx, cur in enumerate(loops):
                pair, j = cur.pair, cur.j
                if pair == 0 and j == 0:
                    add_vproj_fillers()
                if pair < 3 and j == 0:
                    add_qkproj_fillers(pair + 1)
                # out-proj chunk n becomes legal once pair3's j=n norm is
                # flushed; that happens at the flush point of (pair3, j+1)
                hook = (
                    (lambda j=j: add_yjob_fillers(j - 1))
                    if (pair == 3 and j >= 1) else None
                )
                if pair == 3 and j == 3:
                    # hold back two out-proj fillers: they give the PE
                    # work while the final norm's reciprocal chain runs
                    FILLER_RESERVE[0] = 2
                nxt = loops[idx + 1] if idx + 1 < len(loops) else None
                ocps, rrp = run_attention(cur, nxt, 0 if idx == 0 else 3,
                                          after_flush=hook)
                pending_norms.append(
                    lambda pair=pair, j=j, ocps=ocps, rrp=rrp:
                    norm_part2(pair, j, ocps, rrp)
                )

            FILLER_RESERVE[0] = 0
            while fillers:
                pop_filler()
            flush_norms()
            add_yjob_fillers(3)
            while fillers:
                pop_filler()

    nc.compile()
    return nc


def kernel(x, w_qkv, b_qkv, w_out, b_out):
    global _CACHE, LAST_RESULT
    x = np.asarray(x, np.float32)
    w_qkv = np.asarray(w_qkv, np.float32)
    b_qkv = np.asarray(b_qkv, np.float32)
    w_out = np.asarray(w_out, np.float32)
    b_out = np.asarray(b_out, np.float32)

    # the v-bias add costs 32 DVE ops per core; build without it when the
    # bias is identically zero (rebuilds if ever called with a real bias)
    use_vbias = bool(np.any(b_qkv[2048:3072]))
    if _CACHE is None or _CACHE[0] != use_vbias:
        _CACHE = (use_vbias, _build(use_vbias))
    nc = _CACHE[1]

    tri_c = np.triu(np.ones((128, 128), np.float32), 1).astype(NPBF16)
    negid_c = (NEG * np.eye(128, dtype=np.float32)).astype(NPBF16)
    ones2_c = np.zeros((128, 128), np.float32)
    ones2_c[0, 0:64] = 1.0
    ones2_c[1, 64:128] = 1.0
    ones2_c = ones2_c.astype(NPBF16)

    in_maps = []
    for core in range(8):
        b = core // 2
        g = core % 2
        sl = slice(g * 512, (g + 1) * 512)
        wq = w_qkv[:, 0:1024][:, sl] * SCALE
        wk = w_qkv[:, 1024:2048][:, sl]
        wv = w_qkv[:, 2048:3072][:, sl]
        wqkv_c = np.ascontiguousarray(
            np.concatenate([wq, wk, wv], axis=1).astype(NPBF16)
        )
        bq = b_qkv[0:1024][sl] * SCALE
        bk = b_qkv[1024:2048][sl]
        bqk_c = np.ascontiguousarray(
            np.concatenate([bq, bk]).reshape(CK, 128).T.astype(np.float32)
        )
        # bv2[p, pair] = v-bias of head (2*pair + p//64), dim p%64
        bv_core = b_qkv[2048:3072][sl].reshape(HC, 64)
        bv2_c = np.ascontiguousarray(
            bv_core.reshape(4, 2, 64).transpose(0, 1, 2).reshape(4, 128).T
            .astype(np.float32)
        )
        in_maps.append(
            {
                "xT": np.ascontiguousarray(x[b].T.astype(NPBF16)),
                "wqkv": wqkv_c,
                "bqk": bqk_c,
                "bv2": bv2_c,
                "wout": np.ascontiguousarray(w_out[sl, :].astype(NPBF16)),
                "tri": tri_c,
                "negid": negid_c,
                "ones2": ones2_c,
            }
        )

    res = run_bass_kernel_spmd(nc, in_maps, core_ids=list(range(8)), trace=TRACE)
    LAST_RESULT = res

    out = np.empty((B, T, C), np.float32)
    for b in range(B):
        acc = res.results[2 * b]["yT"].astype(np.float32) + res.results[
            2 * b + 1
        ]["yT"].astype(np.float32)
        out[b] = acc.T + b_out[None, :]
    return out
